# revision 8
# baseline (speedup 1.0000x reference)
# Trainium2 Bass kernel for the 4-branch cross-attention block.
#
# Problem: N=4 batches, L1=L2=1024, D=512, H=8 heads of 64.
#   q1,k1,v1 = proj(input1); q2,k2,v2 = proj(input2)
#   four attention branches (q1k1v1, q1k2v2, q2k1v1, q2k2v2), masked softmax
#   over the key axis, outputs averaged pairwise.
#
# Sharding: 8 cores = 4 batches x 2 head-groups (4 heads each). SPMD — one
# program, per-core data.
#
# v2 design notes (vs the 395 us baseline):
#  - The scalar engine's exp over 16.8M logits (~146 us) is the hard floor;
#    the PE must overlap it at the full 2.4 GHz p-state, which requires a
#    continuously-busy tensor engine (idle gaps drop it to 1.2 GHz).
#  - Attention runs in the transposed "ST" layout (keys on partitions):
#      ST = K @ Q^T, P = exp(ST + key_mask_bias), O^T = [V|1]^T @ P.
#    Both q-sides stream against one kz stationary per (head, kt); PV lags
#    one kt behind QK so the PE never waits on the activation directly.
#  - Host pre-casts x and W to fp16 (no device-side CASTs, half the DMA).
#  - kz (zero-padded per-head K stationaries) is written directly from the
#    k-projection PSUM as 4 big [64,1024] copies per side.
#  - Normalization: denominator row rides along the acc->SBUF copy, the
#    reciprocal runs in a [128,8] layout (DVE recip cost scales with free
#    size), the 1/s row is broadcast over partitions by a rank-1 PE matmul
#    (deferred one branch to hide the DMA round-trip latency), and the
#    final output muls/adds run on the otherwise-idle Pool engine.

import sys

sys.path.insert(0, "/opt/trn_rl_repo")

import numpy as np

import concourse.bacc as bacc
import concourse.mybir as mybir
import concourse.tile as tile
from concourse.bass_utils import run_bass_kernel_spmd

F32 = mybir.dt.float32
F32R = mybir.dt.float32r
F16 = mybir.dt.float16
BF16 = mybir.dt.bfloat16
EXP = mybir.ActivationFunctionType.Exp

L = 1024  # sequence length (both sides)
D = 512  # hidden
NB = 4  # batches
HPG = 4  # heads per core (head group)
HD = 64  # head size
OG = HPG * HD  # output channels per core = 256
KT = L // 128  # 8 key tiles
DT = D // 128  # 4 contraction tiles for projections
INF = 10000.0

_NC = None  # cached compiled program
TRACE = False  # set by test harness to capture an NTFF profile
LAST_RESULT = None  # full BassKernelResults of the last run (for profiling)


def _tt(pool, shape, dtype, tag):
    return pool.tile(shape, dtype, tag=tag, name=tag)


def _install_ntff_hook():
    # antenv.axon_hooks is absent in this image; provide it so
    # run_bass_kernel_spmd(trace=True) can capture NTFF profiles.
    import types, contextlib, ctypes

    if "antenv.axon_hooks" in sys.modules:
        return
    lib = ctypes.CDLL("/opt/axon/libaxon_pjrt.so")
    lib.axon_start_nrt_profile.argtypes = [
        ctypes.POINTER(ctypes.c_int64),
        ctypes.c_size_t,
    ]
    lib.axon_start_nrt_profile.restype = ctypes.c_int64
    lib.axon_stop_nrt_profile.argtypes = [ctypes.c_char_p]
    lib.axon_stop_nrt_profile.restype = ctypes.c_int64

    @contextlib.contextmanager
    def _hook(output_dir, device_ids):
        import jax

        jax.devices()
        if device_ids:
            ids = (ctypes.c_int64 * len(device_ids))(*device_ids)
            rc = lib.axon_start_nrt_profile(ids, len(device_ids))
        else:
            rc = lib.axon_start_nrt_profile(None, 0)
        if rc != 0:
            raise RuntimeError(f"axon_start_nrt_profile rc={rc}")
        try:
            yield
        finally:
            n = lib.axon_stop_nrt_profile(str(output_dir).encode())
            print(f"ntff profile: {n} file(s) in {output_dir}", file=sys.stderr)

    mod = types.ModuleType("antenv.axon_hooks")
    mod.get_axon_ntff_profile_hook = lambda: _hook
    mod.set_axon_ntff_profile_hook = lambda h: None
    sys.modules["antenv.axon_hooks"] = mod


def _build():
    nc = bacc.Bacc("TRN2", target_bir_lowering=False, debug=False, num_devices=8)

    x1T = nc.declare_dram_parameter("x1T", [D, L], F16, isOutput=False)
    x2T = nc.declare_dram_parameter("x2T", [D, L], F16, isOutput=False)
    ws = {}
    for wn in ("wq1", "wk1", "wv1", "wq2", "wk2", "wv2"):
        ws[wn] = nc.declare_dram_parameter(wn, [D, OG], F16, isOutput=False)
    bias1 = nc.declare_dram_parameter("bias1", [128, KT], F32, isOutput=False)
    bias2 = nc.declare_dram_parameter("bias2", [128, KT], F32, isOutput=False)
    hm1 = nc.declare_dram_parameter("hm1", [128, KT], F32, isOutput=False)
    hm2 = nc.declare_dram_parameter("hm2", [128, KT], F32, isOutput=False)
    out1T = nc.declare_dram_parameter("out1T", [OG, L], F32, isOutput=True)
    out2T = nc.declare_dram_parameter("out2T", [OG, L], F32, isOutput=True)

    with tile.TileContext(nc) as tc:
        with (
            tc.tile_pool(name="pers", bufs=1) as pers,
            tc.tile_pool(name="pt", bufs=4) as ptp,
            tc.tile_pool(name="oTs", bufs=4) as oTsp,
            tc.tile_pool(name="sm", bufs=2) as smp,
            tc.tile_pool(name="tmp", bufs=2) as tmpp,
            tc.tile_pool(name="st", bufs=2, space="PSUM") as stp,
            tc.tile_pool(name="acc", bufs=2, space="PSUM") as accp,
        ):
            # ---- input DMAs (already f16 on host), in first-use order so
            # the k1 projection can start after the first 8 transfers ----
            x_r = {1: [], 2: []}
            w_r = {wn: [] for wn in ws}

            def load_x(side):
                dram = {1: x1T, 2: x2T}[side]
                for dk in range(DT):
                    t = _tt(pers, [128, L], F16, f"x{side}_{dk}")
                    nc.sync.dma_start(t[:], dram[dk * 128 : (dk + 1) * 128, :])
                    x_r[side].append(t)

            def load_w(wn):
                for dk in range(DT):
                    t = _tt(pers, [128, OG], F16, f"{wn}_{dk}")
                    nc.sync.dma_start(t[:], ws[wn][dk * 128 : (dk + 1) * 128, :])
                    w_r[wn].append(t)

            load_x(1)
            load_w("wk1")
            load_w("wq1")
            load_x(2)
            load_w("wq2")
            load_w("wv1")
            load_w("wk2")
            load_w("wv2")

            b_sb = {}
            for qs, dram in ((1, bias1), (2, bias2)):
                b = _tt(pers, [128, KT], F32, f"bias{qs}")
                nc.sync.dma_start(b[:], dram[:])
                b_sb[qs] = b
            hm_sb = {}
            for qs, dram in ((1, hm1), (2, hm2)):
                h = _tt(pers, [128, KT], F32, f"hm{qs}")
                nc.sync.dma_start(h[:], dram[:])
                hm_sb[qs] = h
            # ---- persistent device tensors ----
            # kz: zero-padded per-(head, kt) K stationaries, [128, 4*8*128]
            kz = {}
            for ks in (1, 2):
                z = _tt(pers, [128, HPG * KT * 128], F16, f"kz{ks}")
                nc.vector.memset(z[:], 0.0)
                kz[ks] = z
            # qT: [og, L] moving operands, 2 tiles of [128, L] per side
            qT = {1: [], 2: []}
            # v in natural layout with ones column: [128, HPG, 65] per l-tile
            v_e = {1: [], 2: []}
            # output accumulators (SBUF, written by Pool)
            outacc = {
                qs: [_tt(pers, [HD, L], F32, f"out{qs}_{i}") for i in range(HPG)]
                for qs in (1, 2)
            }

            # ---- projection emitters ----
            def proj_k(ks):
                w = w_r[f"wk{ks}"]
                for ot in range(2):
                    ps = _tt(stp, [128, L], F32, "st")
                    for dk in range(DT):
                        for nh in range(2):
                            nc.tensor.matmul(
                                ps[:, nh * 512 : (nh + 1) * 512],
                                w[dk][:, ot * 128 : (ot + 1) * 128],
                                x_r[ks][dk][:, nh * 512 : (nh + 1) * 512],
                                start=(dk == 0),
                                stop=(dk == DT - 1),
                            )
                    # scatter straight into kz: head 2*ot rows 0:64, head
                    # 2*ot+1 rows 64:128, each a contiguous [64, 1024] block.
                    for half in range(2):
                        hh = 2 * ot + half
                        po = half * 64
                        nc.vector.tensor_copy(
                            kz[ks][po : po + 64, hh * L : (hh + 1) * L],
                            ps[po : po + 64, :],
                        )

            def proj_q(qs):
                w = w_r[f"wq{qs}"]
                for ot in range(2):
                    ps = _tt(stp, [128, L], F32, "st")
                    for dk in range(DT):
                        for nh in range(2):
                            nc.tensor.matmul(
                                ps[:, nh * 512 : (nh + 1) * 512],
                                w[dk][:, ot * 128 : (ot + 1) * 128],
                                x_r[qs][dk][:, nh * 512 : (nh + 1) * 512],
                                start=(dk == 0),
                                stop=(dk == DT - 1),
                            )
                    t = _tt(pers, [128, L], F16, f"q{qs}T_{ot}")
                    nc.vector.tensor_copy(t[:], ps[:])
                    qT[qs].append(t)

            def proj_v(side, lts):
                w = w_r[f"wv{side}"]
                for lt in lts:
                    ps = _tt(stp, [128, L], F32, "st")
                    for dk in range(DT):
                        nc.tensor.matmul(
                            ps[:, 0:OG],
                            x_r[side][dk][:, lt * 128 : (lt + 1) * 128],
                            w[dk][:],
                            start=(dk == 0),
                            stop=(dk == DT - 1),
                        )
                    t = _tt(pers, [128, HPG, HD + 1], BF16, f"v{side}_{lt}")
                    nc.vector.tensor_copy(
                        t[:, :, 0:HD], ps[:, 0:OG].rearrange("p (h d) -> p h d", h=HPG)
                    )
                    nc.vector.memset(t[:, :, HD : HD + 1], 1.0)
                    v_e[side].append(t)

            # ---- one attention branch: head h against key-side ks, both
            # q-sides streamed against the shared kz stationary; PV lags one
            # kt behind QK so the PE never waits on the activation. ----
            def branch(h, ks):
                acc = {qs: _tt(accp, [HD + 1, L], F32, "acc") for qs in (1, 2)}
                pt_prev = None
                for kt in range(KT):
                    blk = h * KT + kt
                    lhsT = kz[ks][:, blk * 128 : (blk + 1) * 128]
                    sts = {}
                    for qs in (1, 2):
                        st = _tt(stp, [128, L], F32, "st")
                        for nh in range(2):
                            nc.tensor.matmul(
                                st[:, nh * 512 : (nh + 1) * 512],
                                lhsT,
                                qT[qs][h // 2][:, nh * 512 : (nh + 1) * 512],
                                start=True,
                                stop=True,
                            )
                        sts[qs] = st
                    pts = {}
                    for qs in (1, 2):
                        pt = _tt(ptp, [128, L], BF16, "pt")
                        nc.scalar.activation(
                            pt[:], sts[qs][:], EXP, bias=b_sb[ks][:, kt : kt + 1]
                        )
                        pts[qs] = pt
                    if pt_prev is not None:
                        _pv(h, ks, acc, pt_prev)
                    pt_prev = (kt, pts)
                _pv(h, ks, acc, pt_prev)

                # normalization front half: free the acc banks quickly by
                # copying [o; s] to SBUF (both copies BEFORE the DMA-blocked
                # reciprocal chain so the DVE queue releases both accs), then
                # run the reciprocal in a [128, 8] layout and broadcast the
                # 0.5*mask/s row over 64 partitions with a stride-0 DMA.
                oTs = {}
                bcs = {}
                for qs in (1, 2):
                    o = _tt(oTsp, [HD + 1, L], F32, "oTs")
                    nc.vector.tensor_copy(o[:], acc[qs][:])
                    oTs[qs] = o
                s128s = {}
                for qs in (1, 2):
                    s128 = _tt(smp, [128, KT], F32, "s128")
                    nc.sync.dma_start(s128[:], oTs[qs][HD : HD + 1, :])
                    s128s[qs] = s128
                for qs in (1, 2):
                    r128 = _tt(smp, [128, KT], F32, "r128")
                    nc.vector.reciprocal(r128[:], s128s[qs][:])
                    rm128 = _tt(smp, [128, KT], F32, "rm128")
                    nc.vector.tensor_mul(rm128[:], r128[:], hm_sb[qs][:])
                    sr = _tt(smp, [1, L], F32, "srm")
                    nc.sync.dma_start(sr[:], rm128[:])
                    bc = _tt(smp, [HD, L], F32, "bc")
                    nc.sync.dma_start(
                        bc[:], sr[0:1, None, :].to_broadcast((1, HD, L))
                    )
                    bcs[qs] = bc

                def finish():
                    # per-branch output mul on DVE; ks-combine add on the
                    # otherwise-idle Pool engine. Deferred one branch so the
                    # s -> 1/s -> broadcast DMA chain latency is hidden.
                    for qs in (1, 2):
                        oslice = outacc[qs][h][:]
                        if ks == 1:
                            nc.vector.tensor_mul(
                                oslice, oTs[qs][0:HD, :], bcs[qs][:]
                            )
                        else:
                            t = _tt(tmpp, [HD, L], F32, "tmp")
                            nc.vector.tensor_mul(t[:], oTs[qs][0:HD, :], bcs[qs][:])
                            nc.gpsimd.tensor_add(oslice, oslice, t[:])
                            nc.sync.dma_start(
                                {1: out1T, 2: out2T}[qs][h * HD : (h + 1) * HD, :],
                                oslice,
                            )

                return finish

            def _pv(h, ks, acc, pt_prev):
                kt, pts = pt_prev
                vt = v_e[ks][kt][:, h, :]
                for qs in (1, 2):
                    for nh in range(2):
                        nc.tensor.matmul(
                            acc[qs][:, nh * 512 : (nh + 1) * 512],
                            vt,
                            pts[qs][:, nh * 512 : (nh + 1) * 512],
                            start=(kt == 0),
                            stop=(kt == KT - 1),
                        )

            # ---- emission schedule ----
            proj_k(1)
            proj_q(1)
            proj_q(2)
            proj_v(1, range(KT))
            proj_k(2)
            proj_v(2, range(KT))

            pending = None
            for ks in (1, 2):
                for h in range(HPG):
                    fin = branch(h, ks)
                    if pending is not None:
                        pending()
                    pending = fin
            pending()

    nc.compile()
    return nc


def kernel(**inputs):
    global _NC
    if _NC is None:
        _NC = _build()

    input1 = np.asarray(inputs["input1"], dtype=np.float32)
    input2 = np.asarray(inputs["input2"], dtype=np.float32)
    mask1 = np.asarray(inputs["mask1"], dtype=np.float32)
    mask2 = np.asarray(inputs["mask2"], dtype=np.float32)
    W = {k: np.asarray(inputs[k], dtype=np.float32) for k in
         ("Wq1", "Wk1", "Wv1", "Wq2", "Wk2", "Wv2")}

    in_maps = []
    for core in range(8):
        b, hg = core // 2, core % 2
        og = slice(hg * OG, (hg + 1) * OG)
        m = {
            "x1T": np.ascontiguousarray(input1[b].T.astype(np.float16)),
            "x2T": np.ascontiguousarray(input2[b].T.astype(np.float16)),
            "bias1": np.ascontiguousarray(
                ((mask1[b] - 1.0) * INF).reshape(KT, 128).T
            ),
            "bias2": np.ascontiguousarray(
                ((mask2[b] - 1.0) * INF).reshape(KT, 128).T
            ),
            # [128, 8] layout matching the s-row DMA reshape (partition-major)
            "hm1": np.ascontiguousarray((0.5 * mask1[b]).reshape(128, KT)),
            "hm2": np.ascontiguousarray((0.5 * mask2[b]).reshape(128, KT)),
        }
        for wn in ("q1", "k1", "v1", "q2", "k2", "v2"):
            m["w" + wn] = np.ascontiguousarray(
                W["W" + wn[0] + wn[1]].T[:, og].astype(np.float16)
            )
        in_maps.append(m)

    global LAST_RESULT
    if TRACE:
        _install_ntff_hook()
    res = run_bass_kernel_spmd(_NC, in_maps, list(range(8)), trace=TRACE)
    LAST_RESULT = res

    output1 = np.empty((NB, L, D), dtype=np.float32)
    output2 = np.empty((NB, L, D), dtype=np.float32)
    for core in range(8):
        b, hg = core // 2, core % 2
        og = slice(hg * OG, (hg + 1) * OG)
        output1[b, :, og] = res.results[core]["out1T"].T
        output2[b, :, og] = res.results[core]["out2T"].T
    return (output1, output2)


# revision 10
# speedup vs baseline: 1.3161x; 1.3161x over previous
# Trainium2 Bass kernel for the 4-branch cross-attention block.
#
# Problem: N=4 batches, L1=L2=1024, D=512, H=8 heads of 64.
#   q1,k1,v1 = proj(input1); q2,k2,v2 = proj(input2)
#   four attention branches (q1k1v1, q1k2v2, q2k1v1, q2k2v2), masked softmax
#   over the key axis, outputs averaged pairwise.
#
# Sharding: 8 cores = 4 batches x 2 head-groups (4 heads each). SPMD — one
# program, per-core data.
#
# v2 design notes (vs the 395 us baseline):
#  - The scalar engine's exp over 16.8M logits (~146 us) is the hard floor;
#    the PE must overlap it at the full 2.4 GHz p-state, which requires a
#    continuously-busy tensor engine (idle gaps drop it to 1.2 GHz).
#  - Attention runs in the transposed "ST" layout (keys on partitions):
#      ST = K @ Q^T, P = exp(ST + key_mask_bias), O^T = [V|1]^T @ P.
#    Both q-sides stream against one kz stationary per (head, kt); PV lags
#    one kt behind QK so the PE never waits on the activation directly.
#  - Host pre-casts x and W to fp16 (no device-side CASTs, half the DMA).
#  - kz (zero-padded per-head K stationaries) is written directly from the
#    k-projection PSUM as 4 big [64,1024] copies per side.
#  - Normalization: denominator row rides along the acc->SBUF copy, the
#    reciprocal runs in a [128,8] layout (DVE recip cost scales with free
#    size), the 1/s row is broadcast over partitions by a rank-1 PE matmul
#    (deferred one branch to hide the DMA round-trip latency), and the
#    final output muls/adds run on the otherwise-idle Pool engine.

import sys

sys.path.insert(0, "/opt/trn_rl_repo")

import numpy as np

import concourse.bacc as bacc
import concourse.mybir as mybir
import concourse.tile as tile
from concourse.bass_utils import run_bass_kernel_spmd

F32 = mybir.dt.float32
F32R = mybir.dt.float32r
F16 = mybir.dt.float16
BF16 = mybir.dt.bfloat16
EXP = mybir.ActivationFunctionType.Exp

L = 1024  # sequence length (both sides)
D = 512  # hidden
NB = 4  # batches
HPG = 4  # heads per core (head group)
HD = 64  # head size
OG = HPG * HD  # output channels per core = 256
KT = L // 128  # 8 key tiles
DT = D // 128  # 4 contraction tiles for projections
INF = 10000.0

_NC = None  # cached compiled program
TRACE = False  # set by test harness to capture an NTFF profile
LAST_RESULT = None  # full BassKernelResults of the last run (for profiling)


def _tt(pool, shape, dtype, tag):
    return pool.tile(shape, dtype, tag=tag, name=tag)


def _install_ntff_hook():
    # antenv.axon_hooks is absent in this image; provide it so
    # run_bass_kernel_spmd(trace=True) can capture NTFF profiles.
    import types, contextlib, ctypes

    if "antenv.axon_hooks" in sys.modules:
        return
    lib = ctypes.CDLL("/opt/axon/libaxon_pjrt.so")
    lib.axon_start_nrt_profile.argtypes = [
        ctypes.POINTER(ctypes.c_int64),
        ctypes.c_size_t,
    ]
    lib.axon_start_nrt_profile.restype = ctypes.c_int64
    lib.axon_stop_nrt_profile.argtypes = [ctypes.c_char_p]
    lib.axon_stop_nrt_profile.restype = ctypes.c_int64

    @contextlib.contextmanager
    def _hook(output_dir, device_ids):
        import jax

        jax.devices()
        if device_ids:
            ids = (ctypes.c_int64 * len(device_ids))(*device_ids)
            rc = lib.axon_start_nrt_profile(ids, len(device_ids))
        else:
            rc = lib.axon_start_nrt_profile(None, 0)
        if rc != 0:
            raise RuntimeError(f"axon_start_nrt_profile rc={rc}")
        try:
            yield
        finally:
            n = lib.axon_stop_nrt_profile(str(output_dir).encode())
            print(f"ntff profile: {n} file(s) in {output_dir}", file=sys.stderr)

    mod = types.ModuleType("antenv.axon_hooks")
    mod.get_axon_ntff_profile_hook = lambda: _hook
    mod.set_axon_ntff_profile_hook = lambda h: None
    sys.modules["antenv.axon_hooks"] = mod


def _build():
    nc = bacc.Bacc("TRN2", target_bir_lowering=False, debug=False, num_devices=8)

    x1T = nc.declare_dram_parameter("x1T", [D, L], F16, isOutput=False)
    x2T = nc.declare_dram_parameter("x2T", [D, L], F16, isOutput=False)
    ws = {}
    for wn in ("wq1", "wk1", "wv1", "wq2", "wk2", "wv2"):
        ws[wn] = nc.declare_dram_parameter(wn, [D, OG], F16, isOutput=False)
    bias1 = nc.declare_dram_parameter("bias1", [128, KT], F32, isOutput=False)
    bias2 = nc.declare_dram_parameter("bias2", [128, KT], F32, isOutput=False)
    hm1 = nc.declare_dram_parameter("hm1", [128, KT], F32, isOutput=False)
    hm2 = nc.declare_dram_parameter("hm2", [128, KT], F32, isOutput=False)
    out1T = nc.declare_dram_parameter("out1T", [OG, L], F32, isOutput=True)
    out2T = nc.declare_dram_parameter("out2T", [OG, L], F32, isOutput=True)

    with tile.TileContext(nc) as tc:
        with (
            tc.tile_pool(name="pers", bufs=1) as pers,
            tc.tile_pool(name="pt", bufs=4) as ptp,
            tc.tile_pool(name="oTs", bufs=4) as oTsp,
            tc.tile_pool(name="sm", bufs=2) as smp,
            tc.tile_pool(name="tmp", bufs=2) as tmpp,
            tc.tile_pool(name="st", bufs=2, space="PSUM") as stp,
            tc.tile_pool(name="acc", bufs=2, space="PSUM") as accp,
        ):
            # ---- input DMAs (already f16 on host), in first-use order so
            # the k1 projection can start after the first 8 transfers ----
            x_r = {1: [], 2: []}
            w_r = {wn: [] for wn in ws}

            def load_x(side):
                dram = {1: x1T, 2: x2T}[side]
                for dk in range(DT):
                    t = _tt(pers, [128, L], F16, f"x{side}_{dk}")
                    nc.sync.dma_start(t[:], dram[dk * 128 : (dk + 1) * 128, :])
                    x_r[side].append(t)

            def load_w(wn):
                for dk in range(DT):
                    t = _tt(pers, [128, OG], F16, f"{wn}_{dk}")
                    nc.sync.dma_start(t[:], ws[wn][dk * 128 : (dk + 1) * 128, :])
                    w_r[wn].append(t)

            load_x(1)
            load_w("wk1")
            load_w("wq1")
            load_x(2)
            load_w("wq2")
            load_w("wv1")
            load_w("wk2")
            load_w("wv2")

            b_sb = {}
            for qs, dram in ((1, bias1), (2, bias2)):
                b = _tt(pers, [128, KT], F32, f"bias{qs}")
                nc.sync.dma_start(b[:], dram[:])
                b_sb[qs] = b
            hm_sb = {}
            for qs, dram in ((1, hm1), (2, hm2)):
                h = _tt(pers, [128, KT], F32, f"hm{qs}")
                nc.sync.dma_start(h[:], dram[:])
                hm_sb[qs] = h
            ones64f = _tt(pers, [1, 64], F32, "ones64f")
            nc.vector.memset(ones64f[:], 1.0)
            ones64r = _tt(pers, [1, 64], F32R, "ones64r")
            nc.vector.tensor_copy(ones64r[:], ones64f[:])

            # ---- persistent device tensors ----
            # kz: zero-padded per-(head, kt) K stationaries, [128, 4*8*128]
            kz = {}
            for ks in (1, 2):
                z = _tt(pers, [128, HPG * KT * 128], F16, f"kz{ks}")
                nc.vector.memset(z[:], 0.0)
                kz[ks] = z
            # qT: [og, L] moving operands, 2 tiles of [128, L] per side
            qT = {1: [], 2: []}
            # v in natural layout with ones column: [128, HPG, 65] per l-tile
            v_e = {1: [], 2: []}
            # output accumulators (SBUF, written by Pool)
            outacc = {
                qs: [_tt(pers, [HD, L], F32, f"out{qs}_{i}") for i in range(HPG)]
                for qs in (1, 2)
            }

            # ---- projection emitters ----
            def proj_k(ks):
                w = w_r[f"wk{ks}"]
                for ot in range(2):
                    ps = _tt(stp, [128, L], F32, "st")
                    for dk in range(DT):
                        for nh in range(2):
                            nc.tensor.matmul(
                                ps[:, nh * 512 : (nh + 1) * 512],
                                w[dk][:, ot * 128 : (ot + 1) * 128],
                                x_r[ks][dk][:, nh * 512 : (nh + 1) * 512],
                                start=(dk == 0),
                                stop=(dk == DT - 1),
                            )
                    # scatter straight into kz: head 2*ot rows 0:64, head
                    # 2*ot+1 rows 64:128, each a contiguous [64, 1024] block.
                    for half in range(2):
                        hh = 2 * ot + half
                        po = half * 64
                        nc.vector.tensor_copy(
                            kz[ks][po : po + 64, hh * L : (hh + 1) * L],
                            ps[po : po + 64, :],
                        )

            def proj_q(qs):
                w = w_r[f"wq{qs}"]
                for ot in range(2):
                    ps = _tt(stp, [128, L], F32, "st")
                    for dk in range(DT):
                        for nh in range(2):
                            nc.tensor.matmul(
                                ps[:, nh * 512 : (nh + 1) * 512],
                                w[dk][:, ot * 128 : (ot + 1) * 128],
                                x_r[qs][dk][:, nh * 512 : (nh + 1) * 512],
                                start=(dk == 0),
                                stop=(dk == DT - 1),
                            )
                    t = _tt(pers, [128, L], F16, f"q{qs}T_{ot}")
                    nc.vector.tensor_copy(t[:], ps[:])
                    qT[qs].append(t)

            def proj_v(side, lts):
                w = w_r[f"wv{side}"]
                for lt in lts:
                    ps = _tt(stp, [128, L], F32, "st")
                    for dk in range(DT):
                        nc.tensor.matmul(
                            ps[:, 0:OG],
                            x_r[side][dk][:, lt * 128 : (lt + 1) * 128],
                            w[dk][:],
                            start=(dk == 0),
                            stop=(dk == DT - 1),
                        )
                    t = _tt(pers, [128, HPG, HD + 1], BF16, f"v{side}_{lt}")
                    nc.vector.tensor_copy(
                        t[:, :, 0:HD], ps[:, 0:OG].rearrange("p (h d) -> p h d", h=HPG)
                    )
                    nc.vector.memset(t[:, :, HD : HD + 1], 1.0)
                    v_e[side].append(t)

            # ---- one attention branch: head h against key-side ks, both
            # q-sides streamed against the shared kz stationary; PV lags one
            # kt behind QK so the PE never waits on the activation. ----
            def branch(h, ks):
                acc = {qs: _tt(accp, [HD + 1, L], F32, "acc") for qs in (1, 2)}
                pt_prev = None
                for kt in range(KT):
                    blk = h * KT + kt
                    lhsT = kz[ks][:, blk * 128 : (blk + 1) * 128]
                    sts = {}
                    for qs in (1, 2):
                        st = _tt(stp, [128, L], F32, "st")
                        for nh in range(2):
                            nc.tensor.matmul(
                                st[:, nh * 512 : (nh + 1) * 512],
                                lhsT,
                                qT[qs][h // 2][:, nh * 512 : (nh + 1) * 512],
                                start=True,
                                stop=True,
                            )
                        sts[qs] = st
                    pts = {}
                    for qs in (1, 2):
                        pt = _tt(ptp, [128, L], BF16, "pt")
                        nc.scalar.activation(
                            pt[:], sts[qs][:], EXP, bias=b_sb[ks][:, kt : kt + 1]
                        )
                        pts[qs] = pt
                    if pt_prev is not None:
                        _pv(h, ks, acc, pt_prev)
                    pt_prev = (kt, pts)
                _pv(h, ks, acc, pt_prev)

                # normalization front half: free the acc banks quickly by
                # copying [o; s] to SBUF (both copies BEFORE the DMA-blocked
                # reciprocal chain so the DVE queue releases both accs), then
                # run the reciprocal in a [128, 8] layout and broadcast the
                # 0.5*mask/s row over 64 partitions with a stride-0 DMA.
                oTs = {}
                srm = {}
                for qs in (1, 2):
                    o = _tt(oTsp, [HD + 1, L], F32, "oTs")
                    nc.vector.tensor_copy(o[:], acc[qs][:])
                    oTs[qs] = o
                s128s = {}
                for qs in (1, 2):
                    s128 = _tt(smp, [128, KT], F32, "s128")
                    nc.sync.dma_start(s128[:], oTs[qs][HD : HD + 1, :])
                    s128s[qs] = s128
                for qs in (1, 2):
                    r128 = _tt(smp, [128, KT], F32, "r128")
                    nc.vector.reciprocal(r128[:], s128s[qs][:])
                    rm128 = _tt(smp, [128, KT], F32R, "rm128")
                    nc.vector.tensor_mul(rm128[:], r128[:], hm_sb[qs][:])
                    sr = _tt(smp, [1, L], F32R, "srm")
                    nc.sync.dma_start(sr[:], rm128[:])
                    srm[qs] = sr

                def finish():
                    # rank-1 PE broadcast of 0.5*mask/s over 64 partitions
                    # (into a transient st-ring slot), then the per-branch
                    # output mul on DVE with the ks-combine add on Pool.
                    # Deferred one branch so the s -> 1/s chain latency and
                    # the st-slot occupancy are hidden.
                    for qs in (1, 2):
                        bc = _tt(stp, [128, L], F32, "st")
                        for nh in range(2):
                            nc.tensor.matmul(
                                bc[0:HD, nh * 512 : (nh + 1) * 512],
                                ones64r[:],
                                srm[qs][:, nh * 512 : (nh + 1) * 512],
                                start=True,
                                stop=True,
                            )
                        oslice = outacc[qs][h][:]
                        if ks == 1:
                            nc.vector.tensor_mul(oslice, oTs[qs][0:HD, :], bc[0:HD, :])
                        else:
                            t = _tt(tmpp, [HD, L], F32, "tmp")
                            nc.vector.tensor_mul(t[:], oTs[qs][0:HD, :], bc[0:HD, :])
                            nc.gpsimd.tensor_add(oslice, oslice, t[:])
                            nc.sync.dma_start(
                                {1: out1T, 2: out2T}[qs][h * HD : (h + 1) * HD, :],
                                oslice,
                            )

                return finish

            def _pv(h, ks, acc, pt_prev):
                kt, pts = pt_prev
                vt = v_e[ks][kt][:, h, :]
                for qs in (1, 2):
                    for nh in range(2):
                        nc.tensor.matmul(
                            acc[qs][:, nh * 512 : (nh + 1) * 512],
                            vt,
                            pts[qs][:, nh * 512 : (nh + 1) * 512],
                            start=(kt == 0),
                            stop=(kt == KT - 1),
                        )

            # ---- emission schedule ----
            proj_k(1)
            proj_q(1)
            proj_q(2)
            proj_v(1, range(KT))
            proj_k(2)
            proj_v(2, range(KT))

            pending = None
            for ks in (1, 2):
                for h in range(HPG):
                    fin = branch(h, ks)
                    if pending is not None:
                        pending()
                    pending = fin
            pending()

    nc.compile()
    return nc


def kernel(**inputs):
    global _NC
    if _NC is None:
        _NC = _build()

    input1 = np.asarray(inputs["input1"], dtype=np.float32)
    input2 = np.asarray(inputs["input2"], dtype=np.float32)
    mask1 = np.asarray(inputs["mask1"], dtype=np.float32)
    mask2 = np.asarray(inputs["mask2"], dtype=np.float32)
    W = {k: np.asarray(inputs[k], dtype=np.float32) for k in
         ("Wq1", "Wk1", "Wv1", "Wq2", "Wk2", "Wv2")}

    in_maps = []
    for core in range(8):
        b, hg = core // 2, core % 2
        og = slice(hg * OG, (hg + 1) * OG)
        m = {
            "x1T": np.ascontiguousarray(input1[b].T.astype(np.float16)),
            "x2T": np.ascontiguousarray(input2[b].T.astype(np.float16)),
            "bias1": np.ascontiguousarray(
                ((mask1[b] - 1.0) * INF).reshape(KT, 128).T
            ),
            "bias2": np.ascontiguousarray(
                ((mask2[b] - 1.0) * INF).reshape(KT, 128).T
            ),
            # [128, 8] layout matching the s-row DMA reshape (partition-major)
            "hm1": np.ascontiguousarray((0.5 * mask1[b]).reshape(128, KT)),
            "hm2": np.ascontiguousarray((0.5 * mask2[b]).reshape(128, KT)),
        }
        for wn in ("q1", "k1", "v1", "q2", "k2", "v2"):
            m["w" + wn] = np.ascontiguousarray(
                W["W" + wn[0] + wn[1]].T[:, og].astype(np.float16)
            )
        in_maps.append(m)

    global LAST_RESULT
    if TRACE:
        _install_ntff_hook()
    res = run_bass_kernel_spmd(_NC, in_maps, list(range(8)), trace=TRACE)
    LAST_RESULT = res

    output1 = np.empty((NB, L, D), dtype=np.float32)
    output2 = np.empty((NB, L, D), dtype=np.float32)
    for core in range(8):
        b, hg = core // 2, core % 2
        og = slice(hg * OG, (hg + 1) * OG)
        output1[b, :, og] = res.results[core]["out1T"].T
        output2[b, :, og] = res.results[core]["out2T"].T
    return (output1, output2)


# revision 13
# speedup vs baseline: 1.4086x; 1.0704x over previous
# Trainium2 Bass kernel for the 4-branch cross-attention block.
#
# Problem: N=4 batches, L1=L2=1024, D=512, H=8 heads of 64.
#   q1,k1,v1 = proj(input1); q2,k2,v2 = proj(input2)
#   four attention branches (q1k1v1, q1k2v2, q2k1v1, q2k2v2), masked softmax
#   over the key axis, outputs averaged pairwise.
#
# Sharding: 8 cores = 4 batches x 2 head-groups (4 heads each). SPMD — one
# program, per-core data.
#
# v2 design notes (vs the 395 us baseline):
#  - The scalar engine's exp over 16.8M logits (~146 us) is the hard floor;
#    the PE must overlap it at the full 2.4 GHz p-state, which requires a
#    continuously-busy tensor engine (idle gaps drop it to 1.2 GHz).
#  - Attention runs in the transposed "ST" layout (keys on partitions):
#      ST = K @ Q^T, P = exp(ST + key_mask_bias), O^T = [V|1]^T @ P.
#    Both q-sides stream against one kz stationary per (head, kt); PV lags
#    one kt behind QK so the PE never waits on the activation directly.
#  - Host pre-casts x and W to fp16 (no device-side CASTs, half the DMA).
#  - kz (zero-padded per-head K stationaries) is written directly from the
#    k-projection PSUM as 4 big [64,1024] copies per side.
#  - Normalization: denominator row rides along the acc->SBUF copy, the
#    reciprocal runs in a [128,8] layout (DVE recip cost scales with free
#    size), the 1/s row is broadcast over partitions by a rank-1 PE matmul
#    (deferred one branch to hide the DMA round-trip latency), and the
#    final output muls/adds run on the otherwise-idle Pool engine.

import sys

sys.path.insert(0, "/opt/trn_rl_repo")

import numpy as np

import concourse.bacc as bacc
import concourse.mybir as mybir
import concourse.tile as tile
from concourse.bass_utils import run_bass_kernel_spmd

F32 = mybir.dt.float32
F32R = mybir.dt.float32r
F16 = mybir.dt.float16
BF16 = mybir.dt.bfloat16
EXP = mybir.ActivationFunctionType.Exp

L = 1024  # sequence length (both sides)
D = 512  # hidden
NB = 4  # batches
HPG = 4  # heads per core (head group)
HD = 64  # head size
OG = HPG * HD  # output channels per core = 256
KT = L // 128  # 8 key tiles
DT = D // 128  # 4 contraction tiles for projections
INF = 10000.0

_NC = None  # cached compiled program
TRACE = False  # set by test harness to capture an NTFF profile
LAST_RESULT = None  # full BassKernelResults of the last run (for profiling)


def _tt(pool, shape, dtype, tag):
    return pool.tile(shape, dtype, tag=tag, name=tag)


def _install_ntff_hook():
    # antenv.axon_hooks is absent in this image; provide it so
    # run_bass_kernel_spmd(trace=True) can capture NTFF profiles.
    import types, contextlib, ctypes

    if "antenv.axon_hooks" in sys.modules:
        return
    lib = ctypes.CDLL("/opt/axon/libaxon_pjrt.so")
    lib.axon_start_nrt_profile.argtypes = [
        ctypes.POINTER(ctypes.c_int64),
        ctypes.c_size_t,
    ]
    lib.axon_start_nrt_profile.restype = ctypes.c_int64
    lib.axon_stop_nrt_profile.argtypes = [ctypes.c_char_p]
    lib.axon_stop_nrt_profile.restype = ctypes.c_int64

    @contextlib.contextmanager
    def _hook(output_dir, device_ids):
        import jax

        jax.devices()
        if device_ids:
            ids = (ctypes.c_int64 * len(device_ids))(*device_ids)
            rc = lib.axon_start_nrt_profile(ids, len(device_ids))
        else:
            rc = lib.axon_start_nrt_profile(None, 0)
        if rc != 0:
            raise RuntimeError(f"axon_start_nrt_profile rc={rc}")
        try:
            yield
        finally:
            n = lib.axon_stop_nrt_profile(str(output_dir).encode())
            print(f"ntff profile: {n} file(s) in {output_dir}", file=sys.stderr)

    mod = types.ModuleType("antenv.axon_hooks")
    mod.get_axon_ntff_profile_hook = lambda: _hook
    mod.set_axon_ntff_profile_hook = lambda h: None
    sys.modules["antenv.axon_hooks"] = mod


def _build():
    nc = bacc.Bacc("TRN2", target_bir_lowering=False, debug=False, num_devices=8)

    x1T = nc.declare_dram_parameter("x1T", [D, L], F16, isOutput=False)
    x2T = nc.declare_dram_parameter("x2T", [D, L], F16, isOutput=False)
    ws = {}
    for wn in ("wq1", "wk1", "wv1", "wq2", "wk2", "wv2"):
        ws[wn] = nc.declare_dram_parameter(wn, [D, OG], F16, isOutput=False)
    bias1 = nc.declare_dram_parameter("bias1", [128, KT], F32, isOutput=False)
    bias2 = nc.declare_dram_parameter("bias2", [128, KT], F32, isOutput=False)
    hm1 = nc.declare_dram_parameter("hm1", [128, KT], F32, isOutput=False)
    hm2 = nc.declare_dram_parameter("hm2", [128, KT], F32, isOutput=False)
    out1T = nc.declare_dram_parameter("out1T", [OG, L], F32, isOutput=True)
    out2T = nc.declare_dram_parameter("out2T", [OG, L], F32, isOutput=True)

    with tile.TileContext(nc) as tc:
        with (
            tc.tile_pool(name="pers", bufs=1) as pers,
            tc.tile_pool(name="pt", bufs=4) as ptp,
            tc.tile_pool(name="oTs", bufs=4) as oTsp,
            tc.tile_pool(name="sm", bufs=2) as smp,
            tc.tile_pool(name="tmp", bufs=2) as tmpp,
            tc.tile_pool(name="st", bufs=2, space="PSUM") as stp,
            tc.tile_pool(name="acc", bufs=2, space="PSUM") as accp,
        ):
            # ---- input DMAs (already f16 on host), in first-use order so
            # the k1 projection can start after the first 8 transfers ----
            x_r = {1: [], 2: []}
            w_r = {wn: [] for wn in ws}

            def load_x(side):
                dram = {1: x1T, 2: x2T}[side]
                for dk in range(DT):
                    t = _tt(pers, [128, L], F16, f"x{side}_{dk}")
                    nc.sync.dma_start(t[:], dram[dk * 128 : (dk + 1) * 128, :])
                    x_r[side].append(t)

            def load_w(wn):
                for dk in range(DT):
                    t = _tt(pers, [128, OG], F16, f"{wn}_{dk}")
                    nc.sync.dma_start(t[:], ws[wn][dk * 128 : (dk + 1) * 128, :])
                    w_r[wn].append(t)

            load_x(1)
            load_w("wk1")
            load_w("wq1")
            load_x(2)
            load_w("wq2")
            load_w("wv1")
            load_w("wk2")
            load_w("wv2")

            b_sb = {}
            for qs, dram in ((1, bias1), (2, bias2)):
                b = _tt(pers, [128, KT], F32, f"bias{qs}")
                nc.sync.dma_start(b[:], dram[:])
                b_sb[qs] = b
            hm_sb = {}
            for qs, dram in ((1, hm1), (2, hm2)):
                h = _tt(pers, [128, KT], F32, f"hm{qs}")
                nc.sync.dma_start(h[:], dram[:])
                hm_sb[qs] = h
            ones64f = _tt(pers, [1, 64], F32, "ones64f")
            nc.vector.memset(ones64f[:], 1.0)
            ones64r = _tt(pers, [1, 64], F32R, "ones64r")
            nc.vector.tensor_copy(ones64r[:], ones64f[:])

            # ---- persistent device tensors ----
            # kz: zero-padded per-(head, kt) K stationaries, [128, 4*8*128]
            kz = {}
            for ks in (1, 2):
                z = _tt(pers, [128, HPG * KT * 128], F16, f"kz{ks}")
                nc.vector.memset(z[:], 0.0)
                kz[ks] = z
            # qT: [og, L] moving operands, 2 tiles of [128, L] per side
            qT = {1: [], 2: []}
            # v in natural layout with ones column: [128, HPG, 65] per l-tile
            v_e = {1: [], 2: []}
            # output accumulators (SBUF, written by Pool)
            outacc = {
                qs: [_tt(pers, [HD, L], F32, f"out{qs}_{i}") for i in range(HPG)]
                for qs in (1, 2)
            }

            # ---- projection emitters ----
            def proj_k(ks):
                w = w_r[f"wk{ks}"]
                for ot in range(2):
                    ps = _tt(stp, [128, L], F32, "st")
                    for dk in range(DT):
                        for nh in range(2):
                            nc.tensor.matmul(
                                ps[:, nh * 512 : (nh + 1) * 512],
                                w[dk][:, ot * 128 : (ot + 1) * 128],
                                x_r[ks][dk][:, nh * 512 : (nh + 1) * 512],
                                start=(dk == 0),
                                stop=(dk == DT - 1),
                            )
                    # scatter straight into kz: head 2*ot rows 0:64, head
                    # 2*ot+1 rows 64:128, each a contiguous [64, 1024] block.
                    for half in range(2):
                        hh = 2 * ot + half
                        po = half * 64
                        nc.vector.tensor_copy(
                            kz[ks][po : po + 64, hh * L : (hh + 1) * L],
                            ps[po : po + 64, :],
                        )

            def proj_q(qs):
                w = w_r[f"wq{qs}"]
                for ot in range(2):
                    ps = _tt(stp, [128, L], F32, "st")
                    for dk in range(DT):
                        for nh in range(2):
                            nc.tensor.matmul(
                                ps[:, nh * 512 : (nh + 1) * 512],
                                w[dk][:, ot * 128 : (ot + 1) * 128],
                                x_r[qs][dk][:, nh * 512 : (nh + 1) * 512],
                                start=(dk == 0),
                                stop=(dk == DT - 1),
                            )
                    t = _tt(pers, [128, L], F16, f"q{qs}T_{ot}")
                    nc.vector.tensor_copy(t[:], ps[:])
                    qT[qs].append(t)

            def proj_v(side, lts):
                w = w_r[f"wv{side}"]
                for lt in lts:
                    ps = _tt(stp, [128, L], F32, "st")
                    for dk in range(DT):
                        nc.tensor.matmul(
                            ps[:, 0:OG],
                            x_r[side][dk][:, lt * 128 : (lt + 1) * 128],
                            w[dk][:],
                            start=(dk == 0),
                            stop=(dk == DT - 1),
                        )
                    t = _tt(pers, [128, HPG, HD + 1], BF16, f"v{side}_{lt}")
                    nc.vector.tensor_copy(
                        t[:, :, 0:HD], ps[:, 0:OG].rearrange("p (h d) -> p h d", h=HPG)
                    )
                    nc.vector.memset(t[:, :, HD : HD + 1], 1.0)
                    v_e[side].append(t)

            # ---- attention: one continuous software pipeline across all
            # (branch, kt) steps: QK(n) || exp(n-ish) || PV(n-1), so the PE
            # never sees a branch-boundary refill bubble. ----
            def norm_front(h, ks, acc):
                # free the acc banks quickly by copying [o; s] to SBUF (both
                # copies BEFORE the DMA-blocked reciprocal chain), then run
                # the reciprocal in a [128, 8] layout.
                oTs = {}
                srm = {}
                for qs in (1, 2):
                    o = _tt(oTsp, [HD + 1, L], F32, "oTs")
                    nc.vector.tensor_copy(o[:], acc[qs][:])
                    oTs[qs] = o
                s128s = {}
                for qs in (1, 2):
                    s128 = _tt(smp, [128, KT], F32, "s128")
                    nc.sync.dma_start(s128[:], oTs[qs][HD : HD + 1, :])
                    s128s[qs] = s128
                for qs in (1, 2):
                    r128 = _tt(smp, [128, KT], F32, "r128")
                    nc.vector.reciprocal(r128[:], s128s[qs][:])
                    rm128 = _tt(smp, [128, KT], F32R, "rm128")
                    nc.vector.tensor_mul(rm128[:], r128[:], hm_sb[qs][:])
                    sr = _tt(smp, [1, L], F32R, "srm")
                    nc.sync.dma_start(sr[:], rm128[:])
                    srm[qs] = sr

                def finish():
                    # rank-1 PE broadcast of 0.5*mask/s over 64 partitions
                    # (into a transient st-ring slot), then the per-branch
                    # output mul on DVE with the ks-combine add on Pool.
                    # Deferred one branch so the s -> 1/s chain latency and
                    # the st-slot occupancy are hidden.
                    for qs in (1, 2):
                        bc = _tt(stp, [128, L], F32, "st")
                        for nh in range(2):
                            nc.tensor.matmul(
                                bc[0:HD, nh * 512 : (nh + 1) * 512],
                                ones64r[:],
                                srm[qs][:, nh * 512 : (nh + 1) * 512],
                                start=True,
                                stop=True,
                            )
                        oslice = outacc[qs][h][:]
                        if ks == 1:
                            nc.vector.tensor_mul(oslice, oTs[qs][0:HD, :], bc[0:HD, :])
                        else:
                            t = _tt(tmpp, [HD, L], F32, "tmp")
                            nc.vector.tensor_mul(t[:], oTs[qs][0:HD, :], bc[0:HD, :])
                            nc.gpsimd.tensor_add(oslice, oslice, t[:])
                            nc.sync.dma_start(
                                {1: out1T, 2: out2T}[qs][h * HD : (h + 1) * HD, :],
                                oslice,
                            )

                return finish

            def attention(sched):
                accs = {}
                prev = None
                pending = None
                steps = [
                    (bi, h, ks, kt)
                    for bi, (h, ks) in enumerate(sched)
                    for kt in range(KT)
                ]
                for bi, h, ks, kt in steps:
                    if kt == 0:
                        accs[bi] = {
                            qs: _tt(accp, [HD + 1, L], F32, "acc") for qs in (1, 2)
                        }
                    blk = h * KT + kt
                    lhsT = kz[ks][:, blk * 128 : (blk + 1) * 128]
                    sts = {}
                    for qs in (1, 2):
                        st = _tt(stp, [128, L], F32, "st")
                        for nh in range(2):
                            nc.tensor.matmul(
                                st[:, nh * 512 : (nh + 1) * 512],
                                lhsT,
                                qT[qs][h // 2][:, nh * 512 : (nh + 1) * 512],
                                start=True,
                                stop=True,
                            )
                        sts[qs] = st
                    pts = {}
                    for qs in (1, 2):
                        pt = _tt(ptp, [128, L], BF16, "pt")
                        nc.scalar.activation(
                            pt[:], sts[qs][:], EXP, bias=b_sb[ks][:, kt : kt + 1]
                        )
                        pts[qs] = pt
                    if prev is not None:
                        pending = _retire(prev, accs, pending)
                    prev = (bi, h, ks, kt, pts)
                pending = _retire(prev, accs, pending)
                if pending is not None:
                    pending()

            def _retire(prev, accs, pending):
                bi, h, ks, kt, pts = prev
                vt = v_e[ks][kt][:, h, :]
                for qs in (1, 2):
                    for nh in range(2):
                        nc.tensor.matmul(
                            accs[bi][qs][:, nh * 512 : (nh + 1) * 512],
                            vt,
                            pts[qs][:, nh * 512 : (nh + 1) * 512],
                            start=(kt == 0),
                            stop=(kt == KT - 1),
                        )
                if kt == KT - 1:
                    fin = norm_front(h, ks, accs.pop(bi))
                    if pending is not None:
                        pending()
                    return fin
                return pending

            # ---- emission schedule ----
            proj_k(1)
            proj_q(1)
            proj_q(2)
            proj_v(1, range(KT))
            proj_k(2)
            proj_v(2, range(KT))

            sched = [(h, ks) for ks in (1, 2) for h in range(HPG)]
            attention(sched)

    nc.compile()
    return nc


def kernel(**inputs):
    global _NC
    if _NC is None:
        _NC = _build()

    input1 = np.asarray(inputs["input1"], dtype=np.float32)
    input2 = np.asarray(inputs["input2"], dtype=np.float32)
    mask1 = np.asarray(inputs["mask1"], dtype=np.float32)
    mask2 = np.asarray(inputs["mask2"], dtype=np.float32)
    W = {k: np.asarray(inputs[k], dtype=np.float32) for k in
         ("Wq1", "Wk1", "Wv1", "Wq2", "Wk2", "Wv2")}

    in_maps = []
    for core in range(8):
        b, hg = core // 2, core % 2
        og = slice(hg * OG, (hg + 1) * OG)
        m = {
            "x1T": np.ascontiguousarray(input1[b].T.astype(np.float16)),
            "x2T": np.ascontiguousarray(input2[b].T.astype(np.float16)),
            "bias1": np.ascontiguousarray(
                ((mask1[b] - 1.0) * INF).reshape(KT, 128).T
            ),
            "bias2": np.ascontiguousarray(
                ((mask2[b] - 1.0) * INF).reshape(KT, 128).T
            ),
            # [128, 8] layout matching the s-row DMA reshape (partition-major)
            "hm1": np.ascontiguousarray((0.5 * mask1[b]).reshape(128, KT)),
            "hm2": np.ascontiguousarray((0.5 * mask2[b]).reshape(128, KT)),
        }
        for wn in ("q1", "k1", "v1", "q2", "k2", "v2"):
            m["w" + wn] = np.ascontiguousarray(
                W["W" + wn[0] + wn[1]].T[:, og].astype(np.float16)
            )
        in_maps.append(m)

    global LAST_RESULT
    if TRACE:
        _install_ntff_hook()
    res = run_bass_kernel_spmd(_NC, in_maps, list(range(8)), trace=TRACE)
    LAST_RESULT = res

    output1 = np.empty((NB, L, D), dtype=np.float32)
    output2 = np.empty((NB, L, D), dtype=np.float32)
    for core in range(8):
        b, hg = core // 2, core % 2
        og = slice(hg * OG, (hg + 1) * OG)
        output1[b, :, og] = res.results[core]["out1T"].T
        output2[b, :, og] = res.results[core]["out2T"].T
    return (output1, output2)
